# revision 42
# baseline (speedup 1.0000x reference)
"""BERT-NER (12-layer BERT-base + token compaction + classifier) on 8 TRN2 cores.

Data-parallel over batch: 16 sequences -> 2 per core, weights replicated.

v2 design:
- All GEMMs in bf16 (host-cast weights); fp32 PSUM accumulation. Halves HBM
  traffic and SBUF footprint vs fp32; rel err ~4e-3 (tolerance 2e-2).
- Deferred LayerNorm: the residual stream is kept PRE-LN ("y"). LN gain g is
  folded into consumer weights host-side (Wq,Wk,W1); consumers run directly
  on y while LN stats (mean/std/rstd) compute concurrently on DVE/ACT. The
  per-token correction lands as one k=2 matmul per output group
  (lhsT=[c1;c2], rhs=[-mean;std]) and the rstd scale as one DVE multiply at
  PSUM evacuation. PE never waits for LayerNorm.
- True x (post-LN) is materialized off the critical path for the residual
  adds, V projection and classifier.
- Attention softmax row-sums via an all-ones [128,64] matmul that lands the
  sums already broadcast over the DH partitions (ctx scale needs no extra
  broadcast step).
- Projection biases ride in ACT-bias evacuations or the c2 correction row,
  not PE rank-1 matmuls.
"""

import os
import sys

for _p in ("/opt/trn_rl_repo", "/root/.axon_site/_ro/trn_rl_repo"):
    if os.path.isdir(_p) and _p not in sys.path:
        sys.path.insert(0, _p)

import numpy as np
import ml_dtypes

import concourse.bass as bass
import concourse.mybir as mybir
import concourse.tile as tile
from concourse.tile import add_dep_helper
from concourse import bacc, bass_utils

F32 = mybir.dt.float32
F32R = mybir.dt.float32r
BF16 = mybir.dt.bfloat16
I32 = mybir.dt.int32
AF = mybir.ActivationFunctionType
ALU = mybir.AluOpType
NPBF16 = ml_dtypes.bfloat16

B, S, H, L, A, V, NL = 16, 256, 768, 12, 12, 30522, 9
DH = H // A  # 64
FF = 4 * H  # 3072
NC = 8  # cores
BL = B // NC  # 2 sequences per core
T = BL * S  # 512 tokens per core
KT = H // 128  # 6 k-tiles of the hidden dim
TC = T // 128  # 4 token chunks
BIG = 1_000_000  # OOB dump index for compaction scatter
EPS = 1e-12
ISCALE = 1.0 / float(np.sqrt(DH))

P = 128


def _r(ap):
    return ap.bitcast(F32R)


def _f(ap):
    return ap.bitcast(F32)


def build_nc(repeat=1, n_layers=L):
    nc = bacc.Bacc("TRN2", target_bir_lowering=False, debug=False)

    d_ids = nc.dram_tensor("input_word_ids", [BL, S], I32, kind="ExternalInput")
    d_mask = nc.dram_tensor("input_mask", [BL, S], I32, kind="ExternalInput")
    d_type = nc.dram_tensor("input_type_ids", [BL, S], I32, kind="ExternalInput")
    d_valid = nc.dram_tensor("valid_mask", [BL, S], I32, kind="ExternalInput")
    d_wemb = nc.dram_tensor("word_emb", [V, H], BF16, kind="ExternalInput")
    d_pemb = nc.dram_tensor("pos_emb", [S, H], BF16, kind="ExternalInput")
    d_temb = nc.dram_tensor("type_emb", [2, H], BF16, kind="ExternalInput")
    d_elng = nc.dram_tensor("emb_ln_g", [H], F32, kind="ExternalInput")
    d_elnb = nc.dram_tensor("emb_ln_b", [H], F32, kind="ExternalInput")
    d_Wq = nc.dram_tensor("WqF", [L, H, H], BF16, kind="ExternalInput")
    d_Wk = nc.dram_tensor("WkF", [L, H, H], BF16, kind="ExternalInput")
    d_Wv = nc.dram_tensor("Wv", [L, H, H], BF16, kind="ExternalInput")
    d_Wo = nc.dram_tensor("Wo", [L, H, H], BF16, kind="ExternalInput")
    d_W1 = nc.dram_tensor("W1F", [L, H, FF], BF16, kind="ExternalInput")
    d_W2 = nc.dram_tensor("W2", [L, FF, H], BF16, kind="ExternalInput")
    d_CQ = nc.dram_tensor("CQ", [L, 2, H], BF16, kind="ExternalInput")
    d_CK = nc.dram_tensor("CK", [L, 2, H], BF16, kind="ExternalInput")
    d_C1 = nc.dram_tensor("C1", [L, 2, FF], BF16, kind="ExternalInput")
    d_bv = nc.dram_tensor("bvB", [L, H], BF16, kind="ExternalInput")
    d_bo = nc.dram_tensor("bo", [L, H], F32, kind="ExternalInput")
    d_b2 = nc.dram_tensor("b2", [L, H], F32, kind="ExternalInput")
    d_alg = nc.dram_tensor("attn_ln_g", [L, H], F32, kind="ExternalInput")
    d_alb = nc.dram_tensor("attn_ln_b", [L, H], F32, kind="ExternalInput")
    d_flg = nc.dram_tensor("ffn_ln_g", [L, H], F32, kind="ExternalInput")
    d_flb = nc.dram_tensor("ffn_ln_b", [L, H], F32, kind="ExternalInput")
    d_clsW = nc.dram_tensor("cls_W", [H, NL], BF16, kind="ExternalInput")
    d_clsb = nc.dram_tensor("cls_b", [NL], F32, kind="ExternalInput")
    d_out = nc.dram_tensor("out", [BL, S, NL], F32, kind="ExternalOutput")

    dr = dict(
        ids=d_ids, mask=d_mask, type=d_type, valid=d_valid, wemb=d_wemb,
        pemb=d_pemb, temb=d_temb, elng=d_elng, elnb=d_elnb,
        Wq=d_Wq, Wk=d_Wk, Wv=d_Wv, Wo=d_Wo, W1=d_W1, W2=d_W2,
        CQ=d_CQ, CK=d_CK, C1=d_C1, bv=d_bv, bo=d_bo, b2=d_b2,
        alg=d_alg, alb=d_alb, flg=d_flg, flb=d_flb,
        clsW=d_clsW, clsb=d_clsb, out=d_out,
    )

    with nc.allow_low_precision(reason="bf16 matmul pipeline"), tile.TileContext(
        nc
    ) as tc:
        with (
            tc.tile_pool(name="const", bufs=1) as cpool,
            tc.tile_pool(name="main", bufs=1) as mpool,
            tc.tile_pool(name="wts", bufs=3) as wpool,
            tc.tile_pool(name="w2p", bufs=6) as w2pool,
            tc.tile_pool(name="hrows", bufs=2) as rpool,
            tc.tile_pool(name="hbuf", bufs=3) as hpool,
            tc.tile_pool(name="ebuf", bufs=6) as epool,
            tc.tile_pool(name="small", bufs=2) as spool,
        ):
            pools = dict(c=cpool, m=mpool, w=wpool, w2=w2pool, r=rpool,
                         h=hpool, e=epool, s=spool)
            # ---- constants (device-generated) ----
            ident = cpool.tile([P, P], BF16, tag="ident")
            nc.gpsimd.memset(ident[:], 0.0)
            nc.gpsimd.affine_select(
                out=ident[:], in_=ident[:], compare_op=ALU.not_equal, fill=1.0,
                base=0, pattern=[[-1, P]], channel_multiplier=1,
            )
            ones_f32 = cpool.tile([P, 512], F32, tag="ones_f32")
            nc.gpsimd.memset(ones_f32[:], 1.0)
            ones_bf = cpool.tile([P, 512], BF16, tag="ones_bf")
            nc.gpsimd.memset(ones_bf[:], 1.0)
            ones_col_bf = cpool.tile([P, 1], BF16, tag="ones_col_bf")
            nc.vector.tensor_copy(out=ones_col_bf[:], in_=ones_bf[:, :1])
            ones_col_fr = cpool.tile([P, 1], F32R, tag="ones_col_fr")
            nc.vector.tensor_copy(out=ones_col_fr[:], in_=ones_f32[:, :1])
            # lower-triangular-inclusive: ltri[p, ks, t] = 1 if (ks*128+p) <= t
            ltri_f = cpool.tile([P, 2, S], F32, tag="ltri_f")
            nc.gpsimd.memset(ltri_f[:], 1.0)
            nc.gpsimd.affine_select(
                out=ltri_f[:], in_=ltri_f[:], compare_op=ALU.is_ge, fill=0.0,
                base=0, pattern=[[-P, 2], [1, S]], channel_multiplier=-1,
            )
            ones_row_fr = cpool.tile([1, P], F32R, tag="ones_row_fr")
            nc.vector.tensor_copy(out=ones_row_fr[:], in_=ones_f32[:1, :P])
            c_eps = cpool.tile([1, 1], F32, tag="c_eps")
            nc.gpsimd.memset(c_eps[:], EPS)
            consts = dict(ident=ident, ones_f32=ones_f32, ones_bf=ones_bf,
                          ones_col_bf=ones_col_bf, ones_col_fr=ones_col_fr,
                          ones_row_fr=ones_row_fr, ltri=ltri_f, c_eps=c_eps)

            def body():
                emit_body(nc, tc, pools, consts, dr, n_layers)

            if repeat == 1:
                body()
            else:
                with tc.For_i(0, repeat, 1):
                    body()

    nc.compile()
    return nc


def _load_w(nc, wpool, d_slice, name):
    """Load a [H, N] bf16 DRAM slice as SBUF [128, KT, N] (k-tiles on
    partitions), split across both HWDGE queues."""
    n = d_slice.shape[1]
    w = wpool.tile([P, KT, n], BF16, tag="w_big", name=name)
    src = d_slice.rearrange("(kt p) c -> p kt c", p=P)
    nc.sync.dma_start(w[:, 0:3], src[:, 0:3])
    nc.scalar.dma_start(w[:, 3:6], src[:, 3:6])
    return w


def _bias_col(nc, spool, d_vec, tag):
    """Load [H] f32 DRAM vector as [128, KT] (col m = slice m*128:(m+1)*128)."""
    t = spool.tile([P, KT], F32, tag=tag, name=tag)
    nc.sync.dma_start(t[:], d_vec.rearrange("(kt p) -> p kt", p=P))
    return t


def _row(nc, rpool, d_vec, tag, dtype):
    n = d_vec.shape[0]
    t = rpool.tile([1, n], dtype, tag=tag, name=tag)
    nc.sync.dma_start(t[:], d_vec[None, :])
    return t


def emit_rows(nc, pools, consts, ps_s1, ps_s2, tag):
    """From accumulated ps_s1[0]=sum(y), ps_s2[0]=sum(y^2) rows, produce
    corr_rows bf16 [2,512] (row0=-mean, row1=std) and rstd f32r row.
    Critical path split across DVE and ACT to shorten the serial chain."""
    spool = pools["s"]
    c_eps = consts["c_eps"]
    mean = spool.tile([1, 512], F32, tag="mean", bufs=2, name=f"mean_{tag}")
    nc.vector.tensor_scalar_mul(mean[:], ps_s1[0:1], 1.0 / H)
    e2 = spool.tile([1, 512], F32, tag="e2", bufs=2, name=f"e2_{tag}")
    nc.vector.tensor_scalar_mul(e2[:], ps_s2[0:1], 1.0 / H)
    m2 = spool.tile([1, 512], F32, tag="m2", bufs=2, name=f"m2_{tag}")
    nc.scalar.activation(m2[:], mean[:], AF.Square)
    var = spool.tile([1, 512], F32, tag="var", bufs=2, name=f"var_{tag}")
    nc.vector.tensor_tensor(out=var[:], in0=e2[:], in1=m2[:],
                            op=ALU.subtract)
    stdbf = spool.tile([1, 512], BF16, tag="stdbf", bufs=2,
                       name=f"stdbf_{tag}")
    nc.scalar.activation(stdbf[:], var[:], AF.Sqrt, bias=c_eps[:])
    rstd = spool.tile([1, 512], F32R, tag="rstd", bufs=2, name=f"rstd_{tag}")
    nc.vector.reciprocal(rstd[:], stdbf[:])
    corr = spool.tile([2, 512], BF16, tag="corr", bufs=2, name=f"corr_{tag}")
    nc.vector.tensor_scalar_mul(corr[0:1], mean[:], -1.0)
    # partition 1 is not writable by DVE/ACT (32-aligned bases only);
    # a SBUF->SBUF DMA has no such restriction
    nc.sync.dma_start(corr[1:2], stdbf[:])
    return corr, rstd


def make_bcasts(nc, pools, consts, corr, rstd, tag):
    """Broadcast -mean (bf16) and rstd (f32) rows across 128 partitions.

    Returns (negmeanB, rstdB, emit): the tiles exist immediately so later
    closures can capture them; emit(ps1, ps2) emits the two rank-1 matmuls
    + copies into caller-chosen scratch PSUM, letting the caller place them
    AFTER independent PE work in the queue (the rows chain is still
    completing on DVE/ACT at that point)."""
    mpool = pools["m"]
    ones_bf = consts["ones_bf"]
    negmeanB = mpool.tile([P, 512], BF16, tag="negmeanB", bufs=2,
                          name=f"nmB_{tag}")
    rstdB = mpool.tile([P, 512], F32, tag="rstdB", bufs=2, name=f"rB_{tag}")

    def emit(ps_b, ps_b2):
        nc.tensor.matmul(ps_b[:], ones_bf[:1, :P], corr[0:1], start=True,
                         stop=True)
        nc.vector.tensor_copy(out=negmeanB[:], in_=ps_b[:])
        nc.tensor.matmul(ps_b2[:], consts["ones_row_fr"][:1, :P], rstd[:],
                         start=True, stop=True)
        nc.vector.tensor_copy(out=rstdB[:], in_=ps_b2[:])

    return negmeanB, rstdB, emit


def make_materialize(nc, pools, y, negmeanB, rstdB, g_col, b_col, tag):
    """True x = ((y - mean) * rstd) * g + b, bf16 feature-major.

    Returns (x_tile, closures): each closure emits one tile's ops; the
    caller drains them interleaved with other work so the DVE queue never
    sees a 12-op burst at a layer boundary."""
    mpool, hpool = pools["m"], pools["h"]
    x = mpool.tile([P, KT, 512], BF16, tag=tag, bufs=2, name=tag)

    def mat_tile(m):
        t1 = hpool.tile([P, 512], F32, tag="mat_t1", bufs=2, name="mat_t1")
        nc.vector.tensor_tensor(out=t1[:], in0=y[:, m], in1=negmeanB[:],
                                op=ALU.add)
        nc.vector.tensor_tensor(out=t1[:], in0=t1[:], in1=rstdB[:],
                                op=ALU.mult)
        nc.scalar.activation(x[:, m], t1[:], AF.Identity,
                             scale=g_col[:, m : m + 1],
                             bias=b_col[:, m : m + 1])

    return x, [lambda m=m: mat_tile(m) for m in range(KT)]


def emit_body(nc, tc, pools, consts, dr, n_layers):
    cpool, mpool, wpool, w2pool = (
        pools["c"], pools["m"], pools["w"], pools["w2"])
    rpool, hpool, epool, spool = (
        pools["r"], pools["h"], pools["e"], pools["s"])
    ident = consts["ident"]
    ones_bf = consts["ones_bf"]
    ones_f32 = consts["ones_f32"]
    ones_col_bf = consts["ones_col_bf"]
    ones_col_fr = consts["ones_col_fr"]
    ltri = consts["ltri"]

    ids_flat = dr["ids"].rearrange("b s -> (b s)")
    type_flat = dr["type"].rearrange("b s -> (b s)")
    mask_flat = dr["mask"].rearrange("b s -> (b s)")
    valid_flat = dr["valid"].rearrange("b s -> (b s)")

    mask01 = cpool.tile([P, TC], F32, tag="mask01", name="mask01")
    maskDH = cpool.tile([P, TC, DH], BF16, tag="maskDH", name="maskDH")
    valid_f = cpool.tile([P, TC], F32, tag="valid_f", name="valid_f")

    # ============ embeddings (token-major, bf16), transpose ============
    xtok = mpool.tile([P, TC, H], BF16, tag="xtok", name="xtok")
    for c in range(TC):
        idt = spool.tile([P, 1], I32, tag="idt", name="idt")
        nc.sync.dma_start(idt[:], ids_flat[c * P : (c + 1) * P, None])
        nc.gpsimd.indirect_dma_start(
            out=xtok[:, c], out_offset=None, in_=dr["wemb"][:, :],
            in_offset=bass.IndirectOffsetOnAxis(ap=idt[:, :1], axis=0),
        )
        tyt = spool.tile([P, 1], I32, tag="tyt", name="tyt")
        nc.sync.dma_start(tyt[:], type_flat[c * P : (c + 1) * P, None])
        temb = hpool.tile([P, H], BF16, tag="temb", bufs=2, name="temb")
        nc.gpsimd.indirect_dma_start(
            out=temb[:], out_offset=None, in_=dr["temb"][:, :],
            in_offset=bass.IndirectOffsetOnAxis(ap=tyt[:, :1], axis=0),
        )
        pemb = hpool.tile([P, H], BF16, tag="pemb", bufs=2, name="pemb")
        cc = c % (S // P)
        nc.sync.dma_start(pemb[:], dr["pemb"][cc * P : (cc + 1) * P, :])
        nc.vector.tensor_tensor(out=xtok[:, c], in0=xtok[:, c], in1=temb[:],
                                op=ALU.add)
        nc.vector.tensor_tensor(out=xtok[:, c], in0=xtok[:, c], in1=pemb[:],
                                op=ALU.add)

        mi = spool.tile([P, 1], I32, tag="mi", name="mi")
        nc.sync.dma_start(mi[:], mask_flat[c * P : (c + 1) * P, None])
        nc.vector.tensor_copy(out=mask01[:, c : c + 1], in_=mi[:])
        nc.vector.tensor_scalar_mul(maskDH[:, c], ones_bf[:, :DH],
                                    mask01[:, c : c + 1])
        vi = spool.tile([P, 1], I32, tag="vi", name="vi")
        nc.sync.dma_start(vi[:], valid_flat[c * P : (c + 1) * P, None])
        nc.vector.tensor_copy(out=valid_f[:, c : c + 1], in_=vi[:])

    # transpose to feature-major y0 + stats
    y = mpool.tile([P, KT, 512], BF16, tag="y_emb", name="y0")
    with tc.tile_pool(name="embtr", bufs=1, space="PSUM") as ppool:
        ps_stat = ppool.tile([1, 512], F32, tag="stat", name="stat_emb",
                             space="PSUM")
        ps_stat2 = ppool.tile([1, 512], F32, tag="stat2", name="stat2_emb",
                              space="PSUM")
        # chunk-major so transposes start as soon as each token chunk's
        # embeddings land
        for c in range(TC):
            for kt in range(KT):
                ps_t = ppool.tile([P, P], BF16, tag="tr", bufs=3, space="PSUM")
                nc.tensor.transpose(
                    out=ps_t[:], in_=xtok[:, c, kt * P : (kt + 1) * P],
                    identity=ident[:])
                nc.vector.tensor_copy(out=y[:, kt, c * P : (c + 1) * P],
                                      in_=ps_t[:])
        for kt in range(KT):
            sq = hpool.tile([P, 512], F32R, tag="sq", bufs=6, name="sq_emb")
            nc.scalar.activation(sq[:], y[:, kt], AF.Square)
            nc.tensor.matmul(ps_stat[0:1], ones_col_bf[:], y[:, kt],
                             start=(kt == 0), stop=(kt == KT - 1))
            nc.tensor.matmul(ps_stat2[0:1], ones_col_fr[:], sq[:],
                             start=(kt == 0), stop=(kt == KT - 1))
        corr, rstd = emit_rows(nc, pools, consts, ps_stat, ps_stat2, "emb")
    negmeanB, rstdB, bc_emit = make_bcasts(nc, pools, consts, corr, rstd,
                                           "emb")
    eg = _bias_col(nc, spool, dr["elng"][:], "eg")
    eb = _bias_col(nc, spool, dr["elnb"][:], "eb")
    x, mat_defer = make_materialize(nc, pools, y, negmeanB, rstdB, eg, eb,
                                    "x_cur")

    # ============ transformer layers ============
    for l in range(n_layers):
        # ---- Q, K projections (deferred-LN: consume y directly) ----
        qT = mpool.tile([P, KT, 512], BF16, tag="qT", name="qT")
        kT = mpool.tile([P, KT, 512], BF16, tag="kT", name="kT")
        with (
            tc.tile_pool(name=f"qk{l}", bufs=4, space="PSUM") as ppool,
            tc.tile_pool(name=f"v{l}", bufs=2, space="PSUM") as vpool,
        ):
            wq = _load_w(nc, wpool, dr["Wq"][l], "w_q")
            wk = _load_w(nc, wpool, dr["Wk"][l], "w_k")
            cq = rpool.tile([2, H], BF16, tag="cq", name="cq")
            nc.sync.dma_start(cq[:], dr["CQ"][l])
            ck = rpool.tile([2, H], BF16, tag="ck", name="ck")
            nc.sync.dma_start(ck[:], dr["CK"][l])
            groups = [(qT, wq, cq, m) for m in range(KT)] + \
                     [(kT, wk, ck, m) for m in range(KT)]
            DEPTH = 4
            open_groups = []

            def flush_one():
                ps, dst, cw, m = open_groups.pop(0)
                nc.tensor.matmul(ps[:], cw[:, m * P : (m + 1) * P], corr[:],
                                 start=False, stop=True)
                nc.vector.tensor_tensor(out=dst[:, m], in0=ps[:], in1=rstdB[:],
                                        op=ALU.mult)
                if mat_defer:
                    mat_defer.pop(0)()

            for gi, (dst, w, cw, m) in enumerate(groups):
                if len(open_groups) == DEPTH:
                    flush_one()
                ps = ppool.tile([P, 512], F32, tag="qk_ps", space="PSUM")
                for kt in range(KT):
                    nc.tensor.matmul(ps[:], w[:, kt, m * P : (m + 1) * P],
                                     y[:, kt], start=(kt == 0), stop=False)
                open_groups.append((ps, dst, cw, m))
                if gi == 0 and bc_emit is not None:
                    ps1 = ppool.tile([P, 512], F32, tag="bc_ps", bufs=1,
                                     space="PSUM")
                    ps2 = ppool.tile([P, 512], F32, tag="bc_ps", bufs=1,
                                     space="PSUM")
                    bc_emit(ps1, ps2)
                    bc_emit = None
            while open_groups:
                flush_one()
            while mat_defer:
                mat_defer.pop(0)()

            # ---- V (token-major, consume materialized x); its PSUM pool
            # is co-allocated with QK's so its banks never collide with the
            # just-freed QK banks (kills the V-start WAR stall) ----
            vsb = mpool.tile([P, TC, A, DH], BF16, tag="vsb", name="vsb")
            bvr = _row(nc, rpool, dr["bv"][l], "bvr", BF16)
            wv = _load_w(nc, wpool, dr["Wv"][l], "w_v")
            for hh in range(2):
                for c in range(TC):
                    ps = vpool.tile([P, 384], F32, tag="v_ps", space="PSUM")
                    for kt in range(KT):
                        nc.tensor.matmul(
                            ps[:], x[:, kt, c * P : (c + 1) * P],
                            wv[:, kt, hh * 384 : (hh + 1) * 384],
                            start=(kt == 0), stop=False)
                    nc.tensor.matmul(ps[:], ones_bf[:1, :P],
                                     bvr[:1, hh * 384 : (hh + 1) * 384],
                                     start=False, stop=True)
                    # fold the key mask into V: masked keys contribute 0
                    nc.vector.tensor_scalar_mul(
                        vsb[:, c, hh * 6 : (hh + 1) * 6, :],
                        ps[:].rearrange("p (h d) -> p h d", d=DH),
                        mask01[:, c : c + 1])

        # ---- attention, head-PAIR at a time (heads 2j,2j+1 share the
        # ctxT[:, j] tile as partition halves 0:64 / 64:128).
        # The key mask rides in maskDH (rowsum lhsT) and in vsb, so exp
        # needs no bias and covers a whole [P,2,S] bank in one op. ----
        ctxT = mpool.tile([P, KT, 512], BF16, tag="ctxT", name="ctxT")
        with tc.tile_pool(name=f"att{l}", bufs=2, space="PSUM") as ppool:
            pairs = [(b, j) for b in range(BL) for j in range(A // 2)]

            def issue_scores(b, j):
                es = []
                for half in range(2):
                    fo = half * DH
                    ps_s = ppool.tile([P, 2, S], F32, tag="s_ps", bufs=4,
                                      space="PSUM")
                    for kc in range(2):
                        nc.tensor.matmul(
                            ps_s[:, kc],
                            kT[fo : fo + DH, j,
                               b * S + kc * P : b * S + (kc + 1) * P],
                            qT[fo : fo + DH, j, b * S : (b + 1) * S],
                            start=True, stop=True)
                    e = epool.tile([P, 2, S], BF16, tag="e_sb", bufs=6,
                                   name="e_sb")
                    nc.scalar.activation(e[:], ps_s[:], AF.Exp, scale=ISCALE)
                    es.append(e)
                return es

            def issue_tail(b, j, es):
                ps_r = ppool.tile([P, S], F32, tag="r_ps", space="PSUM")
                for half in range(2):
                    for kc in range(2):
                        nc.tensor.matmul(
                            ps_r[half * DH : (half + 1) * DH],
                            maskDH[:, b * 2 + kc], es[half][:, kc],
                            start=(kc == 0), stop=(kc == 1))
                bsb = epool.tile([P, S], F32, tag="bsb", bufs=3, name="bsb")
                nc.vector.reciprocal(bsb[:], ps_r[:])
                ps_c = ppool.tile([P, S], F32, tag="c_ps", space="PSUM")
                for half in range(2):
                    h = 2 * j + half
                    for kc in range(2):
                        nc.tensor.matmul(
                            ps_c[half * DH : (half + 1) * DH],
                            vsb[:, b * 2 + kc, h], es[half][:, kc],
                            start=(kc == 0), stop=(kc == 1))
                nc.vector.tensor_tensor(
                    out=ctxT[:, j, b * S : (b + 1) * S],
                    in0=ps_c[:], in1=bsb[:], op=ALU.mult)

            pending = []
            LOOKAHEAD = 2
            for b, j in pairs:
                es = issue_scores(b, j)
                pending.append((b, j, es))
                if len(pending) > LOOKAHEAD:
                    issue_tail(*pending.pop(0))
            while pending:
                issue_tail(*pending.pop(0))

        # ---- output projection + residual; y_attn stats ----
        y_attn = mpool.tile([P, KT, 512], BF16, tag="y_attn", name="y_attn")
        sq_tiles = []
        with (
            tc.tile_pool(name=f"wo{l}", bufs=2, space="PSUM") as ppool,
            tc.tile_pool(name=f"woln{l}", bufs=1, space="PSUM") as lnpool,
        ):
            bo_col = _bias_col(nc, spool, dr["bo"][l], "bo_col")
            wo = _load_w(nc, wpool, dr["Wo"][l], "w_o")
            ps_stat = lnpool.tile([1, 512], F32, tag="stat", name="stat_a",
                                  space="PSUM")
            ps_stat2 = lnpool.tile([1, 512], F32, tag="stat2", name="stat2_a",
                                   space="PSUM")

            def stat_mm(m):
                nc.tensor.matmul(ps_stat[0:1], ones_col_bf[:], y_attn[:, m],
                                 start=(m == 0), stop=(m == KT - 1))
                nc.tensor.matmul(ps_stat2[0:1], ones_col_fr[:],
                                 sq_tiles[m][:],
                                 start=(m == 0), stop=(m == KT - 1))

            for m in range(KT):
                ps = ppool.tile([P, 512], F32, tag="o_ps", space="PSUM")
                for kt in range(KT):
                    nc.tensor.matmul(ps[:], wo[:, kt, m * P : (m + 1) * P],
                                     ctxT[:, kt], start=(kt == 0),
                                     stop=(kt == KT - 1))
                if m > 0:
                    stat_mm(m - 1)
                tmp = hpool.tile([P, 512], BF16, tag="o_tmp", bufs=2,
                                 name="o_tmp")
                nc.scalar.activation(tmp[:], ps[:], AF.Identity,
                                     bias=bo_col[:, m : m + 1])
                nc.vector.tensor_tensor(out=y_attn[:, m], in0=tmp[:],
                                        in1=x[:, m], op=ALU.add)
                sq = hpool.tile([P, 512], F32R, tag="sq", bufs=6, name="sq_a")
                nc.scalar.activation(sq[:], y_attn[:, m], AF.Square)
                sq_tiles.append(sq)
            stat_mm(KT - 1)
            corr_a, rstd_a = emit_rows(nc, pools, consts, ps_stat, ps_stat2, f"a{l}")
        negmeanB_a, rstdB_a, bc_emit_a = make_bcasts(nc, pools, consts,
                                                     corr_a, rstd_a, f"a{l}")

        # ---- FFN ----
        y_ffn = mpool.tile([P, KT, 512], BF16, tag="y_ffn", name="y_ffn")
        ag = _bias_col(nc, spool, dr["alg"][l], "ag")
        ab = _bias_col(nc, spool, dr["alb"][l], "ab")
        sq_tiles = []
        with (
            tc.tile_pool(name=f"ffa{l}", bufs=1, space="PSUM") as papool,
            tc.tile_pool(name=f"ffh{l}", bufs=2, space="PSUM") as ppool,
        ):
            c1w = rpool.tile([2, FF], BF16, tag="c1w", bufs=1, name="c1w")
            nc.sync.dma_start(c1w[:], dr["C1"][l])
            b2_col = _bias_col(nc, spool, dr["b2"][l], "b2_col")
            ps_y = [papool.tile([P, 512], F32, tag=f"acc{m}",
                                name=f"ps_y{l}_{m}", space="PSUM")
                    for m in range(KT)]
            hsbs = {}  # c -> hsb tile awaiting FFN2

            def stage1(ps_h, c):
                # corr + rstd-scale + gelu; frees the ps_h bank
                nc.tensor.matmul(ps_h[:], c1w[:, c * P : (c + 1) * P],
                                 corr_a[:], start=False, stop=True)
                tmp = hpool.tile([P, 512], F32, tag="h_tmp", bufs=2,
                                 name="h_tmp")
                nc.vector.tensor_tensor(out=tmp[:], in0=ps_h[:],
                                        in1=rstdB_a[:], op=ALU.mult)
                hsb = hpool.tile([P, 512], BF16, tag="h_sb", name="hsb")
                nc.scalar.activation(hsb[:], tmp[:], AF.Gelu_apprx_tanh)
                w2 = w2pool.tile([P, H], BF16, tag="w2c", name="w2")
                nc.sync.dma_start(w2[:], dr["W2"][l][c * P : (c + 1) * P, :])
                hsbs[c] = (hsb, w2)

            def stage2(c):
                hsb, w2 = hsbs.pop(c)
                for m in range(KT):
                    nc.tensor.matmul(ps_y[m][:], w2[:, m * P : (m + 1) * P],
                                     hsb[:], start=(c == 0),
                                     stop=(c == FF // P - 1))

            x2 = mpool.tile([P, KT, 512], BF16, tag="x2", bufs=1, name="x2")
            prev1 = None  # (ps_h, c) awaiting stage1
            prev2 = None  # chunk id awaiting stage2
            for q4 in range(4):
                w1 = _load_w(nc, wpool, dr["W1"][l][:, q4 * H : (q4 + 1) * H],
                             "w_1")
                for cc in range(KT):
                    c = q4 * KT + cc
                    ps_h = ppool.tile([P, 512], F32, tag="h_ps", space="PSUM")
                    for kt in range(KT):
                        nc.tensor.matmul(ps_h[:],
                                         w1[:, kt, cc * P : (cc + 1) * P],
                                         y_attn[:, kt], start=(kt == 0),
                                         stop=False)
                    if c == 1:
                        # ps_y accumulators are idle until FFN2 chunk 0;
                        # borrow two as broadcast scratch (start=True on the
                        # first FFN2 matmul resets them)
                        bc_emit_a(ps_y[0], ps_y[1])
                    if prev1 is not None:
                        stage1(*prev1)
                        if prev2 is not None:
                            stage2(prev2)
                        prev2 = prev1[1]
                    prev1 = (ps_h, c)
                    if 8 <= c < 20 and c % 2 == 0:
                        # spread x2 materialization (FFN2 residual) over the
                        # FFN1 chunks so it never bursts the DVE queue
                        m = (c - 8) // 2
                        t1 = hpool.tile([P, 512], F32, tag="mat_t1", bufs=2,
                                        name="mat_t1")
                        nc.vector.tensor_tensor(out=t1[:], in0=y_attn[:, m],
                                                in1=negmeanB_a[:], op=ALU.add)
                        nc.vector.tensor_tensor(out=t1[:], in0=t1[:],
                                                in1=rstdB_a[:], op=ALU.mult)
                        nc.scalar.activation(x2[:, m], t1[:], AF.Identity,
                                             scale=ag[:, m : m + 1],
                                             bias=ab[:, m : m + 1])
            stage1(*prev1)
            stage2(prev2)
            stage2(prev1[1])
            for m in range(KT):
                tmp = hpool.tile([P, 512], BF16, tag="y_tmp", bufs=2,
                                 name="y_tmp")
                nc.scalar.activation(tmp[:], ps_y[m][:], AF.Identity,
                                     bias=b2_col[:, m : m + 1])
                nc.vector.tensor_tensor(out=y_ffn[:, m], in0=tmp[:],
                                        in1=x2[:, m], op=ALU.add)
                sq = hpool.tile([P, 512], F32R, tag="sq", bufs=6, name="sq_f")
                nc.scalar.activation(sq[:], y_ffn[:, m], AF.Square)
                sq_tiles.append(sq)
            ps_stat = ppool.tile([P, 512], F32, tag="h_ps", space="PSUM")
            ps_stat2 = ppool.tile([P, 512], F32, tag="h_ps", space="PSUM")
            for m in range(KT):
                nc.tensor.matmul(ps_stat[0:1, :], ones_col_bf[:], y_ffn[:, m],
                                 start=(m == 0), stop=(m == KT - 1))
                nc.tensor.matmul(ps_stat2[0:1, :], ones_col_fr[:],
                                 sq_tiles[m][:],
                                 start=(m == 0), stop=(m == KT - 1))
            corr, rstd = emit_rows(nc, pools, consts, ps_stat, ps_stat2, f"f{l}")
        negmeanB, rstdB, bc_emit = make_bcasts(nc, pools, consts, corr, rstd,
                                               f"f{l}")
        fg = _bias_col(nc, spool, dr["flg"][l], "fg")
        fb = _bias_col(nc, spool, dr["flb"][l], "fb")
        x, mat_defer = make_materialize(nc, pools, y_ffn, negmeanB, rstdB,
                                        fg, fb, "x_cur")
        y = y_ffn

    # drain the last layer's bcasts + materialization (x feeds the head)
    if bc_emit is not None:
        with tc.tile_pool(name="lastbc", bufs=1, space="PSUM") as bpool:
            ps1 = bpool.tile([P, 512], F32, tag="bc1", space="PSUM")
            ps2 = bpool.tile([P, 512], F32, tag="bc2", space="PSUM")
            bc_emit(ps1, ps2)
            bc_emit = None
    while mat_defer:
        mat_defer.pop(0)()

    # ============ classifier head + softmax + compaction ============
    with tc.tile_pool(name="head", bufs=2, space="PSUM") as ppool:
        clsw = spool.tile([P, KT, NL], BF16, tag="clsw", name="clsw")
        nc.sync.dma_start(clsw[:], dr["clsW"].rearrange("(kt p) c -> p kt c",
                                                        p=P))
        clsb = _row(nc, rpool, dr["clsb"][:], "clsb", F32)

        # uniform pad row: softmax(cls_b), broadcast to 128 partitions
        nmx = spool.tile([1, 1], F32, tag="nmx", name="nmx")
        nc.vector.reduce_max(out=nmx[:], in_=clsb[:], negate=True,
                             axis=mybir.AxisListType.X)
        usum = spool.tile([1, 1], F32, tag="usum", name="usum")
        uex = spool.tile([1, NL], F32, tag="uex", name="uex")
        nc.scalar.activation(uex[:], clsb[:], AF.Exp, bias=nmx[:],
                             accum_out=usum[:])
        urs = spool.tile([1, 1], F32, tag="urs", name="urs")
        nc.vector.reciprocal(urs[:], usum[:])
        uni = spool.tile([1, NL], F32, tag="uni", name="uni")
        nc.vector.tensor_scalar_mul(uni[:], uex[:], urs[:])
        ps_u = ppool.tile([P, NL], F32, tag="u_ps", space="PSUM")
        nc.tensor.matmul(ps_u[:], ones_f32[:1, :P], uni[:], start=True,
                         stop=True)
        uni128 = spool.tile([P, NL], F32, tag="uni128", name="uni128")
        nc.vector.tensor_copy(out=uni128[:], in_=ps_u[:])
        out_flat = dr["out"].rearrange("b s c -> (b s) c")
        prefills = []
        for c in range(TC):
            dma = nc.sync.dma_start(out_flat[c * P : (c + 1) * P, :], uni128[:])
            prefills.append(dma.ins)

        for c in range(TC):
            b = c // (S // P)
            ps_lg = ppool.tile([P, NL], F32, tag="lg_ps", space="PSUM")
            for kt in range(KT):
                nc.tensor.matmul(ps_lg[:], x[:, kt, c * P : (c + 1) * P],
                                 clsw[:, kt], start=(kt == 0), stop=False)
            nc.tensor.matmul(ps_lg[:], ones_f32[:1, :P], clsb[:],
                             start=False, stop=True)
            negmax = spool.tile([P, 1], F32, tag="negmax", name="negmax")
            nc.vector.reduce_max(out=negmax[:], in_=ps_lg[:], negate=True,
                                 axis=mybir.AxisListType.X)
            probs = spool.tile([P, NL], F32, tag="probs", name="probs")
            sm = spool.tile([P, 1], F32, tag="sm", name="sm")
            nc.scalar.activation(probs[:], ps_lg[:], AF.Exp, bias=negmax[:],
                                 accum_out=sm[:])
            rs = spool.tile([P, 1], F32, tag="rs", name="rs")
            nc.vector.reciprocal(rs[:], sm[:])
            nc.vector.tensor_scalar_mul(probs[:], probs[:], rs[:])

            # cumsum of valid over the sequence, sliced to this chunk
            cc = c % (S // P)
            ps_cs = ppool.tile([P, 1], F32, tag="cs_ps", space="PSUM")
            for ks in range(2):
                nc.tensor.matmul(ps_cs[:], ltri[:, ks, cc * P : (cc + 1) * P],
                                 valid_f[:, b * 2 + ks : b * 2 + ks + 1],
                                 start=(ks == 0), stop=(ks == 1))
            # dest = valid ? b*S + csum - 1 : BIG
            dest_f = spool.tile([P, 1], F32, tag="dest_f", name="dest_f")
            nc.vector.tensor_scalar_add(dest_f[:], ps_cs[:],
                                        float(b * S - 1 - BIG))
            nc.vector.tensor_tensor(out=dest_f[:], in0=dest_f[:],
                                    in1=valid_f[:, c : c + 1], op=ALU.mult)
            nc.vector.tensor_scalar_add(dest_f[:], dest_f[:], float(BIG))
            dest_i = spool.tile([P, 1], I32, tag="dest_i", name="dest_i")
            nc.vector.tensor_copy(out=dest_i[:], in_=dest_f[:])

            scat = nc.gpsimd.indirect_dma_start(
                out=out_flat[:, :],
                out_offset=bass.IndirectOffsetOnAxis(ap=dest_i[:, :1], axis=0),
                in_=probs[:],
                in_offset=None,
                bounds_check=T - 1, oob_is_err=False,
            )
            for pf in prefills:
                add_dep_helper(scat.ins, pf,
                               reason="scatter after uniform prefill")


_NC_CACHE = {}


def _get_nc(repeat=1, n_layers=L):
    key = (repeat, n_layers)
    if key not in _NC_CACHE:
        _NC_CACHE[key] = build_nc(repeat=repeat, n_layers=n_layers)
    return _NC_CACHE[key]


def _fold_host(inputs):
    """Host-side precompute: bf16 casts + LN folding into consumer weights."""
    f = {}
    i = {k: np.asarray(v) for k, v in inputs.items()}
    bf = lambda a: np.ascontiguousarray(a.astype(NPBF16))
    f["word_emb"] = bf(i["word_emb"])
    f["pos_emb"] = bf(i["pos_emb"])
    f["type_emb"] = bf(i["type_emb"])
    f["emb_ln_g"] = i["emb_ln_g"].astype(np.float32)
    f["emb_ln_b"] = i["emb_ln_b"].astype(np.float32)
    WqF = np.empty((L, H, H), NPBF16)
    WkF = np.empty((L, H, H), NPBF16)
    W1F = np.empty((L, H, FF), NPBF16)
    CQ = np.empty((L, 2, H), NPBF16)
    CK = np.empty((L, 2, H), NPBF16)
    C1 = np.empty((L, 2, FF), NPBF16)
    for l in range(L):
        g_prev = i["emb_ln_g"] if l == 0 else i["ffn_ln_g"][l - 1]
        b_prev = i["emb_ln_b"] if l == 0 else i["ffn_ln_b"][l - 1]
        wqf = g_prev[:, None] * i["Wq"][l]
        wkf = g_prev[:, None] * i["Wk"][l]
        WqF[l] = wqf.astype(NPBF16)
        WkF[l] = wkf.astype(NPBF16)
        CQ[l] = np.stack([wqf.sum(0),
                          b_prev @ i["Wq"][l] + i["bq"][l]]).astype(NPBF16)
        CK[l] = np.stack([wkf.sum(0),
                          b_prev @ i["Wk"][l] + i["bk"][l]]).astype(NPBF16)
        w1f = i["attn_ln_g"][l][:, None] * i["W1"][l]
        W1F[l] = w1f.astype(NPBF16)
        C1[l] = np.stack([w1f.sum(0),
                          i["attn_ln_b"][l] @ i["W1"][l] + i["b1"][l]]
                         ).astype(NPBF16)
    f["WqF"], f["WkF"], f["W1F"] = WqF, WkF, W1F
    f["CQ"], f["CK"], f["C1"] = CQ, CK, C1
    f["Wv"] = bf(i["Wv"])
    f["Wo"] = bf(i["Wo"])
    f["W2"] = bf(i["W2"])
    f["bvB"] = bf(i["bv"])
    f["bo"] = i["bo"].astype(np.float32)
    f["b2"] = i["b2"].astype(np.float32)
    for k in ("attn_ln_g", "attn_ln_b", "ffn_ln_g", "ffn_ln_b"):
        f[k] = i[k].astype(np.float32)
    f["cls_W"] = bf(i["cls_W"])
    f["cls_b"] = i["cls_b"].astype(np.float32)
    return f


def make_in_maps(inputs):
    per_seq = {}
    for name in ("input_word_ids", "input_mask", "input_type_ids",
                 "valid_mask"):
        per_seq[name] = np.ascontiguousarray(np.asarray(inputs[name]))
    shared = _fold_host(inputs)
    in_maps = []
    for c in range(NC):
        m = dict(shared)
        for name, arr in per_seq.items():
            m[name] = np.ascontiguousarray(arr[c * BL : (c + 1) * BL])
        in_maps.append(m)
    return in_maps


def kernel(**inputs):
    nc = _get_nc()
    in_maps = make_in_maps(inputs)
    res = bass_utils.run_bass_kernel_spmd(nc, in_maps, list(range(NC)))
    out = np.concatenate([res.results[c]["out"] for c in range(NC)], axis=0)
    return out.astype(np.float32)
